# revision 6
# baseline (speedup 1.0000x reference)
"""Trainium2 Bass kernel for nn_CriticNetwork (GRU particle encoder + twin critic MLP).

Sharding: data-parallel over batch, B=1024 -> 128 per core x 8 cores. All
weights replicated. Everything on-core runs in "transposed" layout (feature
dim on SBUF partitions, batch on the free dim) so the sequential GRU scan is
pure weight-stationary matmuls with no per-step transposes:

    pre_t = [Wi_aug]^T x_t + [Wh]^T h_{t-1}       (PSUM accumulation)
    r  = sigmoid(pre_r)
    z' = sigmoid(-pre_z)          (z columns of the weights are pre-negated)
    z  = 1 - z'
    n  = tanh(x_n + r*(h_n + bhn))
    h  = z*h + z'*n

x_t includes the particle-weight channel and a constant ones row that folds
in bi, so x_t^T is a [66, 128] tile; all 256 of them are produced once by
PE transpose-mode matmuls in a pre-phase and kept resident in SBUF.
"""

import os
import sys
import numpy as np

for _p in ("/opt/trn_rl_repo", "/root/.axon_site/_ro/trn_rl_repo"):
    if os.path.isdir(_p) and _p not in sys.path:
        sys.path.insert(0, _p)

import concourse.bass as bass
import concourse.mybir as mybir
import concourse.tile as tile
from concourse import bacc
from concourse.masks import make_identity
from concourse.bass_utils import run_bass_kernel_spmd

AF = mybir.ActivationFunctionType
OP = mybir.AluOpType

B, T, DP, A = 1024, 256, 64, 8
H = 256
HID = 256
C = 2
TIME_NORM = 100.0
NCORES = 8
BS = B // NCORES          # per-core batch = 128
F_AUG = DP + 2            # particles + weight channel + ones(bi) row = 66
G = 3 * H                 # 768 gate columns
TC = 32                   # time chunk for the input transpose pre-phase


class Cfg:
    def __init__(self, mm_dt=mybir.dt.bfloat16, gate_dt=mybir.dt.bfloat16,
                 S=1, t_steps=T, use_gpsimd=True):
        self.mm_dt = mm_dt
        self.gate_dt = gate_dt
        self.S = S                  # independent batch sub-streams
        self.t_steps = t_steps      # reduced for sim debugging
        self.use_gpsimd = use_gpsimd

    def key(self):
        return (str(self.mm_dt), str(self.gate_dt), self.S, self.t_steps,
                self.use_gpsimd)


def build(cfg: Cfg):
    nc = bacc.Bacc("TRN2", target_bir_lowering=False, debug=False,
                   num_devices=NCORES)
    f32 = mybir.dt.float32
    MM = cfg.mm_dt
    GD = cfg.gate_dt
    S = cfg.S
    BW = BS // S            # batch width per sub-stream
    TS = cfg.t_steps

    # ---- DRAM I/O (per-core shapes) ----
    d_part = nc.dram_tensor("particles", [BS, T, DP], f32, kind="ExternalInput")
    d_wts = nc.dram_tensor("weights", [BS, T], f32, kind="ExternalInput")
    d_act = nc.dram_tensor("action", [BS, A], f32, kind="ExternalInput")
    d_time = nc.dram_tensor("time_idx", [BS], f32, kind="ExternalInput")
    d_Wi = nc.dram_tensor("Wi", [DP + 1, G], f32, kind="ExternalInput")
    d_bi = nc.dram_tensor("bi", [G], f32, kind="ExternalInput")
    d_Wh = nc.dram_tensor("Wh", [H, G], f32, kind="ExternalInput")
    d_bhn = nc.dram_tensor("bhn", [H], f32, kind="ExternalInput")
    d_W1 = nc.dram_tensor("W1", [C, H + A + 1, HID], f32, kind="ExternalInput")
    d_b1 = nc.dram_tensor("b1", [C, HID], f32, kind="ExternalInput")
    d_W2 = nc.dram_tensor("W2", [C, HID, HID], f32, kind="ExternalInput")
    d_b2 = nc.dram_tensor("b2", [C, HID], f32, kind="ExternalInput")
    d_W3 = nc.dram_tensor("W3", [C, HID, 1], f32, kind="ExternalInput")
    d_b3 = nc.dram_tensor("b3", [C, 1], f32, kind="ExternalInput")
    d_out = nc.dram_tensor("out", [BS, C], f32, kind="ExternalOutput")

    with tile.TileContext(nc) as tc:
        with (
            tc.tile_pool(name="const", bufs=1) as cp,
            tc.tile_pool(name="state", bufs=1) as sp,
            tc.tile_pool(name="work", bufs=2) as wp,
        ):
            # ---------------- parameter load + layout ----------------
            ident = cp.tile([128, 128], MM, name="ident", tag="ident")
            make_identity(nc, ident[:])

            def load_mm(name, dram_ap, p, f, negate_z=False):
                """DMA a [p, f] fp32 param, cast to MM dtype (negating the
                z-gate columns 256:512 when asked)."""
                stg = wp.tile([p, f], f32, name=f"{name}_stg", tag="pstg")
                nc.sync.dma_start(stg[:, :], dram_ap)
                t_ = cp.tile([p, f], MM, name=name, tag=name)
                if negate_z:
                    nc.vector.tensor_copy(t_[:, 0:H], stg[:, 0:H])
                    nc.vector.tensor_scalar_mul(t_[:, H:2 * H], stg[:, H:2 * H], -1.0)
                    nc.vector.tensor_copy(t_[:, 2 * H:], stg[:, 2 * H:])
                else:
                    nc.vector.tensor_copy(t_[:, :], stg[:, :])
                return t_

            # Wi_aug: rows 0:64 = Wi particle rows, 64 = weight-channel row,
            # 65 = bi row. (bass AP supports row-slices of the dram tensors.)
            wi_stg = wp.tile([F_AUG, G], f32, name="wi_stg", tag="pstg66")
            nc.sync.dma_start(wi_stg[0:DP + 1, :], d_Wi[:, :])
            nc.sync.dma_start(wi_stg[DP + 1:F_AUG, :],
                              d_bi[:].rearrange("(a f) -> a f", a=1))
            wi_mm = cp.tile([F_AUG, G], MM, name="wi_mm", tag="wi_mm")
            nc.vector.tensor_copy(wi_mm[:, 0:H], wi_stg[:, 0:H])
            nc.vector.tensor_scalar_mul(wi_mm[:, H:2 * H], wi_stg[:, H:2 * H], -1.0)
            nc.vector.tensor_copy(wi_mm[:, 2 * H:], wi_stg[:, 2 * H:])

            wh0_mm = load_mm("wh0_mm", d_Wh[0:128, :], 128, G, negate_z=True)
            wh1_mm = load_mm("wh1_mm", d_Wh[128:256, :], 128, G, negate_z=True)

            bhn_sb = cp.tile([128, 2], f32, name="bhn_sb", tag="bhn_sb")
            nc.sync.dma_start(bhn_sb[:, :], d_bhn[:].rearrange("(f p) -> p f", p=128))

            w1k0, w1k1, w1k2, w2k0, w2k1, w3k0, w3k1 = [], [], [], [], [], [], []
            for c in range(C):
                w1k0.append(load_mm(f"w1k0_{c}", d_W1[c, 0:128, :], 128, HID))
                w1k1.append(load_mm(f"w1k1_{c}", d_W1[c, 128:256, :], 128, HID))
                w1k2.append(load_mm(f"w1k2_{c}", d_W1[c, 256:265, :], A + 1, HID))
                w2k0.append(load_mm(f"w2k0_{c}", d_W2[c, 0:128, :], 128, HID))
                w2k1.append(load_mm(f"w2k1_{c}", d_W2[c, 128:256, :], 128, HID))
                w3k0.append(load_mm(f"w3k0_{c}", d_W3[c, 0:128, :], 128, 1))
                w3k1.append(load_mm(f"w3k1_{c}", d_W3[c, 128:256, :], 128, 1))

            b1_sb = cp.tile([128, 2 * C], f32, name="b1_sb", tag="b1_sb")
            b2_sb = cp.tile([128, 2 * C], f32, name="b2_sb", tag="b2_sb")
            for c in range(C):
                nc.sync.dma_start(b1_sb[:, 2 * c:2 * c + 2],
                                  d_b1[c:c + 1, :].rearrange("a (f p) -> p (a f)", p=128))
                nc.sync.dma_start(b2_sb[:, 2 * c:2 * c + 2],
                                  d_b2[c:c + 1, :].rearrange("a (f p) -> p (a f)", p=128))
            b3_sb = cp.tile([1, C], f32, name="b3_sb", tag="b3_sb")
            nc.sync.dma_start(b3_sb[:, :], d_b3[:, :].rearrange("c a -> a c"))

            # critic "extra" k-tile: rows 0:8 action^T, row 8 = time/TIME_NORM
            extra = sp.tile([A + 1, BS], MM, name="extra", tag="extra")
            act_stg = wp.tile([BS, A], f32, name="act_stg", tag="act_stg")
            nc.sync.dma_start(act_stg[:, :], d_act[:, :])
            act_mm = wp.tile([BS, A], MM, name="act_mm", tag="act_mm")
            nc.vector.tensor_copy(act_mm[:, :], act_stg[:, :])
            # engine ops need 32-aligned base partitions; row 8 of `extra` is
            # written via DMA (exempt) from a partition-0 staging row
            time_stg = wp.tile([1, BS], f32, name="time_stg", tag="time_stg")
            nc.sync.dma_start(time_stg[:, :],
                              d_time[:].rearrange("(a f) -> a f", a=1))
            time_mm = wp.tile([1, BS], MM, name="time_mm", tag="time_mm")
            nc.scalar.mul(time_mm[:, :], time_stg[:, :], 1.0 / TIME_NORM)
            nc.sync.dma_start(extra[A:A + 1, :], time_mm[:, :])

            # ---------------- input transpose pre-phase ----------------
            # xT: [66, T*128], column t*128+b holds x_t(b); row 64 = particle
            # weight, row 65 = ones (multiplies the bi row of wi_mm).
            xT = sp.tile([F_AUG, T * BS], MM, name="xT", tag="xT")
            ones_stg = wp.tile([1, TC * BS], MM, name="ones_stg",
                               tag="ones_stg", bufs=1)
            nc.gpsimd.memset(ones_stg[:, :], 1.0)
            for ci in range(T // TC):
                nc.sync.dma_start(
                    xT[DP + 1:F_AUG, ci * TC * BS:(ci + 1) * TC * BS],
                    ones_stg[:, :])

            with tc.tile_pool(name="tpps", bufs=4, space="PSUM") as tpps:
                # action transpose via PE
                aps = tpps.tile([A, BS], MM, name="aps", tag="tp")
                nc.tensor.transpose(aps[:, :], act_mm[:, :], ident[:, :])
                nc.vector.tensor_copy(extra[0:A, :], aps[:, :])

                for ci in range(T // TC):
                    t0 = ci * TC
                    praw = wp.tile([BS, TC, DP], f32, name="praw", tag="praw")
                    wraw = wp.tile([BS, TC], f32, name="wraw", tag="wraw")
                    nc.sync.dma_start(praw[:, :, :], d_part[:, t0:t0 + TC, :])
                    nc.sync.dma_start(wraw[:, :], d_wts[:, t0:t0 + TC])
                    staged = wp.tile([BS, TC, DP + 1], MM, name="staged", tag="staged")
                    nc.vector.tensor_copy(staged[:, :, 0:DP], praw[:, :, :])
                    nc.vector.tensor_copy(staged[:, :, DP], wraw[:, :])
                    for j in range(TC):
                        t_idx = t0 + j
                        tps = tpps.tile([DP + 1, BS], MM, name="tps", tag="tp")
                        nc.tensor.transpose(tps[:, :], staged[:, j, :], ident[:, :])
                        dst = xT[0:DP + 1, t_idx * BS:(t_idx + 1) * BS]
                        if j % 2 == 0:
                            nc.vector.tensor_copy(dst, tps[:, :])
                        else:
                            nc.scalar.copy(dst, tps[:, :])

            # ---------------- GRU scan ----------------
            h_sb = [sp.tile([128, 2 * BW], MM, name=f"h_sb{s}", tag=f"h_sb{s}")
                    for s in range(S)]
            for s in range(S):
                nc.gpsimd.memset(h_sb[s][:, :], 0.0)

            psA_bufs = 3 if S == 1 else 1
            with tc.tile_pool(name="scps", bufs=psA_bufs, space="PSUM") as scps:
                for t in range(TS):
                    for s in range(S):
                        xcol = t * BS + s * BW
                        x_t = xT[:, xcol:xcol + BW]
                        h0 = h_sb[s][:, 0:BW]
                        h1 = h_sb[s][:, BW:2 * BW]

                        psr = scps.tile([128, 2 * BW], f32, name=f"psr{s}",
                                        tag=f"psr{s}", bufs=psA_bufs)
                        psz = scps.tile([128, 2 * BW], f32, name=f"psz{s}",
                                        tag=f"psz{s}", bufs=psA_bufs)
                        psbc = scps.tile([128, 4 * BW], f32, name=f"psbc{s}",
                                         tag=f"psbc{s}", bufs=2)

                        # r / z' pre-activations (z columns pre-negated)
                        for mi, ps in ((0, psr), (1, psr), (2, psz), (3, psz)):
                            col = mi * 128
                            dst = ps[:, (mi % 2) * BW:(mi % 2) * BW + BW]
                            nc.tensor.matmul(dst, wi_mm[:, col:col + 128], x_t,
                                             start=(mi % 2 == 0), stop=False)
                            nc.tensor.matmul(dst, wh0_mm[:, col:col + 128], h0,
                                             start=False, stop=False)
                            nc.tensor.matmul(dst, wh1_mm[:, col:col + 128], h1,
                                             start=False, stop=(mi % 2 == 1))
                        # n gate: psB (h-part) cols 0:2BW, psC (x-part) 2BW:4BW
                        for mi in (4, 5):
                            col = mi * 128
                            dst = psbc[:, (mi - 4) * BW:(mi - 4) * BW + BW]
                            nc.tensor.matmul(dst, wh0_mm[:, col:col + 128], h0,
                                             start=(mi == 4), stop=False)
                            nc.tensor.matmul(dst, wh1_mm[:, col:col + 128], h1,
                                             start=False, stop=False)
                        for mi in (4, 5):
                            col = mi * 128
                            dst = psbc[:, (mi - 2) * BW:(mi - 2) * BW + BW]
                            nc.tensor.matmul(dst, wi_mm[:, col:col + 128], x_t,
                                             start=False, stop=(mi == 5))

                        r_sb = wp.tile([128, 2 * BW], GD, name=f"r_sb{s}",
                                       tag=f"r_sb{s}")
                        zp_sb = wp.tile([128, 2 * BW], GD, name=f"zp_sb{s}",
                                        tag=f"zp_sb{s}")
                        z_sb = wp.tile([128, 2 * BW], GD, name=f"z_sb{s}",
                                       tag=f"z_sb{s}")
                        e1_sb = wp.tile([128, 2 * BW], GD, name=f"e1_sb{s}",
                                        tag=f"e1_sb{s}")
                        t_sb = wp.tile([128, 2 * BW], GD, name=f"t_sb{s}",
                                       tag=f"t_sb{s}")
                        n_sb = wp.tile([128, 2 * BW], GD, name=f"n_sb{s}",
                                       tag=f"n_sb{s}")
                        e2_sb = wp.tile([128, 2 * BW], GD, name=f"e2_sb{s}",
                                        tag=f"e2_sb{s}")

                        nc.scalar.activation(r_sb[:, :], psr[:, :], AF.Sigmoid)
                        nc.scalar.activation(zp_sb[:, :], psz[:, :], AF.Sigmoid)
                        # z = 1 - z'
                        nc.vector.tensor_scalar(z_sb[:, :], zp_sb[:, :],
                                                -1.0, 1.0, OP.mult, OP.add)
                        # e1 = z * h   (off the critical chain)
                        eng = nc.gpsimd if cfg.use_gpsimd else nc.vector
                        eng.tensor_tensor(e1_sb[:, :], z_sb[:, :], h_sb[s][:, :],
                                          OP.mult)
                        # t = (h_n + bhn) * r
                        for m in range(2):
                            nc.vector.scalar_tensor_tensor(
                                t_sb[:, m * BW:(m + 1) * BW],
                                psbc[:, m * BW:(m + 1) * BW],
                                bhn_sb[:, m:m + 1],
                                r_sb[:, m * BW:(m + 1) * BW],
                                OP.add, OP.mult)
                        # n = tanh(x_n + t)
                        u_sb = wp.tile([128, 2 * BW], GD, name=f"u_sb{s}",
                                       tag=f"u_sb{s}")
                        nc.vector.tensor_tensor(u_sb[:, :],
                                                psbc[:, 2 * BW:4 * BW],
                                                t_sb[:, :], OP.add)
                        nc.scalar.activation(n_sb[:, :], u_sb[:, :], AF.Tanh)
                        # h = e1 + z'*n
                        nc.vector.tensor_tensor(e2_sb[:, :], zp_sb[:, :],
                                                n_sb[:, :], OP.mult)
                        nc.vector.tensor_tensor(h_sb[s][:, :], e1_sb[:, :],
                                                e2_sb[:, :], OP.add)

            # ---------------- critic MLPs ----------------
            v_sb = sp.tile([1, C * BS], f32, name="v_sb", tag="v_sb")
            with tc.tile_pool(name="crps", bufs=2, space="PSUM") as crps:
                for s in range(S):
                    h0 = h_sb[s][:, 0:BW]
                    h1 = h_sb[s][:, BW:2 * BW]
                    ex = extra[:, s * BW:(s + 1) * BW]
                    for c in range(C):
                        ps1 = crps.tile([128, 2 * BW], f32, name="ps1", tag="ps1")
                        for m in range(2):
                            col = m * 128
                            dst = ps1[:, m * BW:(m + 1) * BW]
                            nc.tensor.matmul(dst, w1k0[c][:, col:col + 128], h0,
                                             start=(m == 0), stop=False)
                            nc.tensor.matmul(dst, w1k1[c][:, col:col + 128], h1,
                                             start=False, stop=False)
                            nc.tensor.matmul(dst, w1k2[c][:, col:col + 128], ex,
                                             start=False, stop=(m == 1))
                        h1_sb = wp.tile([128, 2 * BW], MM, name="h1_sb", tag="h1_sb")
                        for m in range(2):
                            nc.scalar.activation(h1_sb[:, m * BW:(m + 1) * BW],
                                                 ps1[:, m * BW:(m + 1) * BW],
                                                 AF.Relu,
                                                 bias=b1_sb[:, 2 * c + m:2 * c + m + 1])
                        ps2 = crps.tile([128, 2 * BW], f32, name="ps2", tag="ps2")
                        for m in range(2):
                            col = m * 128
                            dst = ps2[:, m * BW:(m + 1) * BW]
                            nc.tensor.matmul(dst, w2k0[c][:, col:col + 128],
                                             h1_sb[:, 0:BW], start=(m == 0),
                                             stop=False)
                            nc.tensor.matmul(dst, w2k1[c][:, col:col + 128],
                                             h1_sb[:, BW:2 * BW], start=False,
                                             stop=(m == 1))
                        h2_sb = wp.tile([128, 2 * BW], MM, name="h2_sb", tag="h2_sb")
                        for m in range(2):
                            nc.scalar.activation(h2_sb[:, m * BW:(m + 1) * BW],
                                                 ps2[:, m * BW:(m + 1) * BW],
                                                 AF.Relu,
                                                 bias=b2_sb[:, 2 * c + m:2 * c + m + 1])
                        ps3 = crps.tile([1, BW], f32, name="ps3", tag="ps3")
                        nc.tensor.matmul(ps3[:, :], w3k0[c][:, :], h2_sb[:, 0:BW],
                                         start=True, stop=False)
                        nc.tensor.matmul(ps3[:, :], w3k1[c][:, :],
                                         h2_sb[:, BW:2 * BW], start=False,
                                         stop=True)
                        nc.scalar.activation(
                            v_sb[:, c * BS + s * BW:c * BS + (s + 1) * BW],
                            ps3[:, :], AF.Identity, bias=b3_sb[:, c:c + 1])

            for c in range(C):
                nc.sync.dma_start(d_out[:, c].rearrange("(a p) -> a p", a=1),
                                  v_sb[:, c * BS:(c + 1) * BS])

    nc.compile()
    return nc


_CACHE = {}


def get_nc(cfg: Cfg):
    k = cfg.key()
    if k not in _CACHE:
        _CACHE[k] = build(cfg)
    return _CACHE[k]


def shard_inputs(inputs):
    """Full inputs -> list of 8 per-core in_maps (batch-sharded)."""
    rep_keys = ["Wi", "bi", "Wh", "bhn", "W1", "b1", "W2", "b2", "W3", "b3"]
    in_maps = []
    for i in range(NCORES):
        sl = slice(i * BS, (i + 1) * BS)
        m = {
            "particles": np.ascontiguousarray(inputs["particles"][sl], np.float32),
            "weights": np.ascontiguousarray(inputs["weights"][sl], np.float32),
            "action": np.ascontiguousarray(inputs["action"][sl], np.float32),
            "time_idx": np.ascontiguousarray(inputs["time_idx"][sl], np.float32),
        }
        for k in rep_keys:
            m[k] = np.ascontiguousarray(inputs[k], np.float32)
        in_maps.append(m)
    return in_maps


def run(inputs, cfg: Cfg = None, trace: bool = False):
    cfg = cfg or Cfg()
    nc = get_nc(cfg)
    in_maps = shard_inputs(inputs)
    res = run_bass_kernel_spmd(nc, in_maps, core_ids=list(range(NCORES)),
                               trace=trace)
    out = np.concatenate([r["out"] for r in res.results], axis=0)
    return out.astype(np.float32), res


def kernel(**inputs) -> np.ndarray:
    out, _ = run(inputs)
    return out
